# revision 31
# baseline (speedup 1.0000x reference)
"""Causal self-attention (dense transformer block) on 8 Trainium2 NeuronCores.

Sharding: tensor-parallel over heads x data-parallel over batch.
  - 8 cores = 2 batch groups x 4 cores; each core owns 1 batch element and
    4 of the 16 heads (head_dim 64 -> 256 local channels).
  - Host pre-transposes x and the weight slices so the device never has to
    transpose activations (PE contracts along partitions).
  - Each core computes qkv projection for its heads, causal attention in
    "S^T" layout (scores[k, q], k on partitions), and its partial c_proj.
  - Host sums the 4 partials per batch and adds the bias terms.

Math notes:
  - k-bias and v-bias never enter the kernel: the k-bias contribution to the
    scores is constant along the softmax axis (cancels exactly), and the
    v-bias passes through softmax (rows sum to 1) and c_proj into a constant
    output offset w_proj @ b_v, added on host.
  - Softmax skips the max-subtraction pass: scores/8 have |.| <~ 3 for this
    distribution, exp cannot overflow, and the result is mathematically
    identical.
  - attV is computed with V augmented by a ones column, so the softmax
    denominators fall out of the same matmul (row 64 of the PSUM tile).
  - All matmuls run in bf16 (PSUM accumulation stays fp32). fp32r draws
    enough PE power that the HAM throttles the clock to 4/8 duty for half
    the kernel; bf16 needs half the weight-load and DMA bytes and throttles
    far less. Softmax denominators are accumulated and inverted in fp32.

Scheduling notes:
  - Input DMA is t-chunk-major across three queues (v-weights first on the
    gpsimd queue, x 512-column chunks alternating sync/scalar queues) so the
    first V matmul starts after ~1.5 MB instead of ~5 MB of traffic.
  - Work is emitted per head-pair (V, Q^T/K^T, then attention) so the
    second pair's projection matmuls fill the PE gaps while the first
    pair's softmax runs on ScalarE.
  - Softmax denominators are staged on PARTITION 0 (one [1,512] slot per
    head x q-chunk) so gpsimd's partition_broadcast reads them directly --
    no bounce DMA. Each chunk is inverted with reciprocal_approx_fast
    (~5x cheaper than nc.vector.reciprocal) and normalized in place right
    when its attV completes, so the chain never convoys the DVE queue at
    head boundaries.
  - c_proj for the first T/2 rows is emitted between the last head's two
    halves: its matmuls fill the PE gaps left by half-1's softmax waits and
    only half of c_proj remains in the tail.
  - attV PSUM tiles are released right after two cheap copies; the
    normalize-multiply happens later, in place, in SBUF.
"""

import numpy as np
from contextlib import ExitStack

from ml_dtypes import bfloat16

import concourse.bass as bass
import concourse.tile as tile
from concourse import bacc, library_config, mybir
from concourse.bass_utils import run_bass_kernel_spmd

FP32 = mybir.dt.float32
BF16 = mybir.dt.bfloat16
AF = mybir.ActivationFunctionType

B, T_FULL, C = 2, 2048, 1024
H, D = 16, 64
NCORES = 8
CPG = 4          # cores per batch group
HPC = H // CPG   # heads per core = 4
HL = HPC * D     # local channels = 256
NQO = HL // 128  # head pairs per core = 2
CT = C // 128    # contraction tiles = 8


def _nsplit(w):
    """Split width into matmul N-chunks at 512-aligned offsets (a matmul
    output may not cross a PSUM bank line)."""
    chunks = [512] * (w // 512)
    if w % 512:
        chunks.append(w % 512)
    return chunks


def build_bass(T=T_FULL):
    """Emit the SPMD Bass/Tile program for one core (same program, per-core
    data). T must be a multiple of 1024 (two halves per q-range, 512-chunks)."""
    assert T % 1024 == 0
    TT = T // 128          # t-tiles
    HALF = T // 2
    NCH = T // 512         # 512-chunks per head

    nc = bacc.Bacc("TRN2", target_bir_lowering=False, debug=False,
                   num_devices=NCORES)

    xT_d = nc.dram_tensor("xT", [C, T], BF16, kind="ExternalInput")
    wqkvT_d = nc.dram_tensor("wqkvT", [C, 3 * HL], BF16, kind="ExternalInput")
    bq_d = nc.dram_tensor("bq", [HL], FP32, kind="ExternalInput")
    wpT_d = nc.dram_tensor("wpT", [HL, C], BF16, kind="ExternalInput")
    out_d = nc.dram_tensor("out", [T, C], FP32, kind="ExternalOutput")

    with tile.TileContext(nc) as tc, ExitStack() as ctx:
        xt = ctx.enter_context(tc.tile_pool(name="xt", bufs=1))
        wq = ctx.enter_context(tc.tile_pool(name="wq", bufs=1))
        wp = ctx.enter_context(tc.tile_pool(name="wp", bufs=1))
        qk = ctx.enter_context(tc.tile_pool(name="qk", bufs=2 * NQO))
        vv = ctx.enter_context(tc.tile_pool(name="vv", bufs=(TT + 3) // 4))
        es = ctx.enter_context(tc.tile_pool(name="es", bufs=4))
        yt = ctx.enter_context(tc.tile_pool(name="yt", bufs=NQO))
        ob = ctx.enter_context(tc.tile_pool(name="ob", bufs=3))
        bc = ctx.enter_context(tc.tile_pool(name="bc", bufs=3))
        sc = ctx.enter_context(tc.tile_pool(name="sc", bufs=1))
        # PSUM budget (8 banks): qkv/V 2x[128,512]=2, scores/proj 2x[128,1024]=4,
        # attV accumulators 2x[65,512]=2. Separate tags so the second pair's
        # qkv matmuls can fill PE gaps while attention waits on softmax.
        pq = ctx.enter_context(tc.tile_pool(name="pq", bufs=2, space="PSUM"))
        ss = ctx.enter_context(tc.tile_pool(name="ss", bufs=2, space="PSUM"))
        py = ctx.enter_context(tc.tile_pool(name="py", bufs=2, space="PSUM"))

        # ---- inputs -> SBUF ----
        # Few BIG strided DMAs (each dma_start costs ~650ns of queue issue
        # time), ordered so the first V matmul's inputs land first: v-weight
        # slice, then x in 512-column t-chunks (chunk 0 is all the V t-tiles
        # 0-3 and the first QK chunk need), interleaved across two queues.
        wqall = wq.tile([128, CT, 3 * HL], BF16, tag="wq", name="wtile")
        wq_src = wqkvT_d.ap().rearrange("(c p) o -> p c o", p=128)
        nc.sync.dma_start(out=wqall[:, :, 2 * HL:3 * HL],
                          in_=wq_src[:, :, 2 * HL:3 * HL])
        bq_sb = sc.tile([128, NQO], FP32, tag="bq")
        nc.scalar.dma_start(out=bq_sb,
                            in_=bq_d.ap().rearrange("(j p) -> p j", p=128))
        xtall = xt.tile([128, CT, T], BF16, tag="xt", name="xtile")
        x_src = xT_d.ap().rearrange("(c p) t -> p c t", p=128)
        # chunk 0 leads with a tiny [0:128] slice: it is all V t-tile 0
        # needs, and per-queue completion is in-order, so the first matmul
        # fires as soon as ~0.75 MB (v-weights + this slice) has landed
        x_ranges = [(0, 128, nc.sync), (128, 512, nc.sync)]
        for k in range(1, T // 512):
            x_ranges.append((512 * k, 512 * (k + 1),
                             nc.scalar if k % 2 == 1 else nc.sync))
        for lo, hi, eng in x_ranges:
            eng.dma_start(out=xtall[:, :, lo:hi], in_=x_src[:, :, lo:hi])
        # q/k weight slices (needed only once the QK phase starts)
        nc.sync.dma_start(out=wqall[:, :, 0:2 * HL],
                          in_=wq_src[:, :, 0:2 * HL])
        wpall = wp.tile([128, NQO, C], BF16, tag="wp", name="wptile")
        nc.scalar.dma_start(
            out=wpall,
            in_=wpT_d.ap().rearrange("(i p) o -> p i o", p=128))
        wqs = [wqall[:, c] for c in range(CT)]
        xts = [xtall[:, c] for c in range(CT)]
        wps = [wpall[:, i] for i in range(NQO)]

        # ones source for V's denominator column (ACT rounds fp32->bf16)
        ones_sb = sc.tile([128, 4 * HPC], FP32, tag="ones")
        nc.gpsimd.memset(ones_sb, 1.0)
        vts = []
        for g in range((TT + 3) // 4):
            vt = vv.tile([128, 4, HPC, D + 1], BF16, tag="vv", name="vtile")
            nc.scalar.copy(
                vt[:, :, :, D],
                ones_sb.rearrange("p (a b) -> p a b", a=4),
            )
            vts.append(vt)

        qk_tiles = [qk.tile([128, T], BF16, tag="qk", name="qktile")
                    for _ in range(2 * NQO)]
        yts = [yt.tile([128, T], BF16, tag="yt", name="ytile")
               for _ in range(NQO)]
        # softmax denominators, all on partition 0 (one 512-slot per head
        # x q-chunk) so partition_broadcast reads them without a bounce DMA
        dstage = sc.tile([1, HPC * NCH * 512], FP32, tag="dstage")
        dstage_r = sc.tile([1, HPC * NCH * 512], FP32, tag="dstage_r")

        # ---- V for all heads (N=256 keeps the PE at full rate) ----
        for tt in range(TT):
            pv = pq.tile([128, 512], FP32, tag="pq", name="pv")
            for c in range(CT):
                nc.tensor.matmul(
                    pv[:, 0:HL],
                    xts[c][:, tt * 128:(tt + 1) * 128],
                    wqs[c][:, 2 * HL:3 * HL],
                    start=(c == 0), stop=(c == CT - 1),
                )
            nc.vector.tensor_copy(
                vts[tt // 4][:, tt % 4, :, 0:D],
                pv[:, 0:HL].rearrange("p (h d) -> p h d", h=HPC),
            )

        def emit_qk_pair(pair):
            for o in (pair, NQO + pair):
                col0 = o * 128 if o < NQO else HL + (o - NQO) * 128
                for tch in range(T // 512):
                    pt = pq.tile([128, 512], FP32, tag="pq", name="pqk")
                    for c in range(CT):
                        nc.tensor.matmul(
                            pt,
                            wqs[c][:, col0:col0 + 128],
                            xts[c][:, tch * 512:(tch + 1) * 512],
                            start=(c == 0), stop=(c == CT - 1),
                        )
                    dst = qk_tiles[o][:, tch * 512:(tch + 1) * 512]
                    if o < NQO:  # add q bias (per-partition)
                        nc.vector.tensor_scalar_add(dst, pt, bq_sb[:, o:o + 1])
                    else:
                        nc.vector.tensor_copy(dst, pt)

        def emit_cproj(tts):
            for tt in tts:
                po = ss.tile([128, 1024], FP32, tag="ss", name="po")
                for s in range(2):
                    for i in range(NQO):
                        nc.tensor.matmul(
                            po[:, s * 512:(s + 1) * 512],
                            yts[i][:, tt * 128:(tt + 1) * 128],
                            wps[i][:, s * 512:(s + 1) * 512],
                            start=(i == 0), stop=(i == NQO - 1),
                        )
                ot = ob.tile([128, C], FP32, tag="ob", name="otile")
                # split the PSUM->SBUF copy across ACT and DVE so the po
                # tile recycles at matmul pace, and the out DMA across two
                # queues so the halves transfer concurrently
                nc.scalar.copy(ot[:, 0:512], po[:, 0:512])
                nc.vector.tensor_copy(ot[:, 512:1024], po[:, 512:1024])
                nc.sync.dma_start(
                    out=out_d[tt * 128:(tt + 1) * 128, 0:512],
                    in_=ot[:, 0:512])
                nc.scalar.dma_start(
                    out=out_d[tt * 128:(tt + 1) * 128, 512:1024],
                    in_=ot[:, 512:1024])

        def emit_attention_head(pair, h01):
            last_head = (pair == NQO - 1 and h01 == 1)
            hb = 64 * h01
            h = 2 * pair + h01          # local head index 0..3
            qt = qk_tiles[pair]
            kt_tile = qk_tiles[NQO + pair]

            # Normalize-multiplies are deferred by one q-chunk: when they
            # finally run, their broadcast finished a chunk ago, so the
            # in-order DVE queue never blocks ahead of the PSUM-release
            # copies the PE is waiting on.
            pending = []

            def emit_mul(cg):
                bc_t, = [b for c_, b in pending if c_ == cg]
                pending.remove((cg, bc_t))
                dst = yts[pair][hb:hb + 64, cg * 512:(cg + 1) * 512]
                nc.vector.tensor_mul(dst, dst, bc_t[hb:hb + 64, :])

            for half in range(2):
                q0, q1 = half * HALF, (half + 1) * HALF
                py_map = {}
                for kt in range(q1 // 128):
                    qa = max(kt * 128, q0)
                    w = q1 - qa
                    qa0 = (qa // 512) * 512
                    pt = ss.tile([128, 1024], FP32, tag="ss", name="pst")
                    off = 0
                    for cw in _nsplit(w):
                        nc.tensor.matmul(
                            pt[:, off:off + cw],
                            kt_tile[hb:hb + 64, kt * 128:(kt + 1) * 128],
                            qt[hb:hb + 64, qa + off:qa + off + cw],
                            start=True, stop=True,
                        )
                        off += cw
                    es_t = es.tile([128, 1024], BF16, tag="es", name="estile")
                    nc.scalar.activation(
                        es_t[:, qa - qa0:qa - qa0 + w], pt[:, 0:w],
                        AF.Exp, scale=0.125,
                    )
                    if qa == kt * 128:
                        # causal mask: zero exp values where k > q in the
                        # diagonal block (gpsimd, SBUF, off the DVE/PSUM path)
                        nc.gpsimd.affine_select(
                            out=es_t[:, qa - qa0:qa - qa0 + 128],
                            in_=es_t[:, qa - qa0:qa - qa0 + 128],
                            compare_op=mybir.AluOpType.is_ge,
                            fill=0.0, base=0,
                            pattern=[[1, 128]], channel_multiplier=-1,
                        )
                    for cg in range(q0 // 512, q1 // 512):
                        if kt * 128 >= (cg + 1) * 512:
                            continue
                        if cg not in py_map:
                            py_map[cg] = py.tile([65, 512], FP32,
                                                 tag="py", name="pyt")
                        last_kt = min(q1 // 128, (cg + 1) * 4) - 1
                        # clip to causally-valid columns (q >= kt*128)
                        c0 = max(cg * 512, kt * 128)
                        nc.tensor.matmul(
                            py_map[cg][:, c0 - cg * 512:512],
                            vts[kt // 4][:, kt % 4, h, :],
                            es_t[:, c0 - qa0:(cg + 1) * 512 - qa0],
                            start=(kt == 0), stop=(kt == last_kt),
                        )
                        if kt == last_kt:
                            # stage unnormalized y + denominator row, release
                            # the PSUM slot; approx-recip (DVE) and
                            # partition_broadcast (gpsimd, reads partition 0
                            # directly) start now, the in-place multiply runs
                            # one chunk later
                            py_t = py_map[cg]
                            nc.vector.tensor_copy(
                                yts[pair][hb:hb + 64,
                                          cg * 512:(cg + 1) * 512],
                                py_t[0:64, :],
                            )
                            slot = (h * NCH + cg) * 512
                            nc.vector.tensor_copy(
                                dstage[0:1, slot:slot + 512],
                                py_t[64:65, :])
                            nc.vector.reciprocal_approx_fast(
                                out=dstage_r[0:1, slot:slot + 512],
                                in_=dstage[0:1, slot:slot + 512])
                            bc_t = bc.tile([128, 512], FP32, tag="bc",
                                           name="bct")
                            nc.gpsimd.partition_broadcast(
                                bc_t, dstage_r[0:1, slot:slot + 512])
                            if pending:
                                emit_mul(pending[0][0])
                            pending.append((cg, bc_t))
                    if (last_head and half == 1
                            and kt == (NCH - 1) * 4 - 1 and pending):
                        # third q-chunk of the last head is done: flush its
                        # multiply and emit the next c_proj quarter so only
                        # the final quarter is gated by the last chunk
                        while pending:
                            emit_mul(pending[0][0])
                        emit_cproj(range(TT // 2, 3 * TT // 4))
                if last_head and half == 0:
                    # first half of every head's y is final: emit c_proj for
                    # rows 0..T/2 here so its matmuls fill half-1's softmax
                    # waits and only half the c_proj remains in the tail
                    while pending:
                        emit_mul(pending[0][0])
                    emit_cproj(range(TT // 2))
            if last_head or h01 == 0:
                while pending:
                    emit_mul(pending[0][0])
                return None

            def flush():
                while pending:
                    emit_mul(pending[0][0])
            return flush

        # Muls left pending at a pair's end are carried across the next
        # pair's QK emission: they land in the DVE queue AFTER the QK
        # bias-add/copies that recycle the projection PSUM, so the PE never
        # waits on the (gpsimd-gated) normalize chain at phase boundaries.
        carry = None
        for pair in range(NQO):
            emit_qk_pair(pair)
            if carry is not None:
                carry()
            emit_attention_head(pair, 0)
            carry = emit_attention_head(pair, 1)

        emit_cproj(range(3 * TT // 4, TT))

    nc.compile()  # bacc lowering: register allocation, library/ACT table loads
    return nc


_NC_CACHE = {}


def _get_nc(T=T_FULL):
    if T not in _NC_CACHE:
        _NC_CACHE[T] = build_bass(T)
    return _NC_CACHE[T]


def make_in_maps(x, w_attn, b_attn, w_proj, T=T_FULL):
    x = np.ascontiguousarray(np.asarray(x, np.float32))
    w_attn = np.asarray(w_attn, np.float32)
    b_attn = np.asarray(b_attn, np.float32)
    w_proj = np.asarray(w_proj, np.float32)
    xTs = [np.ascontiguousarray(x[b].T.astype(bfloat16))
           for b in range(x.shape[0])]
    in_maps = []
    for core in range(NCORES):
        b, j = core // CPG, core % CPG
        r0 = j * HL
        wq_s = w_attn[r0:r0 + HL]
        wk_s = w_attn[C + r0:C + r0 + HL]
        wv_s = w_attn[2 * C + r0:2 * C + r0 + HL]
        in_maps.append({
            "xT": xTs[b],
            "wqkvT": np.ascontiguousarray(
                np.concatenate([wq_s, wk_s, wv_s], axis=0).T.astype(bfloat16)),
            "bq": np.ascontiguousarray(b_attn[r0:r0 + HL]),
            "wpT": np.ascontiguousarray(
                w_proj[:, r0:r0 + HL].T.astype(bfloat16)),
        })
    return in_maps


def run_device(x, w_attn, b_attn, w_proj, b_proj, T=T_FULL, **spmd_kwargs):
    nc = _get_nc(T)
    in_maps = make_in_maps(x, w_attn, b_attn, w_proj, T)
    res = run_bass_kernel_spmd(nc, in_maps, core_ids=list(range(NCORES)),
                               **spmd_kwargs)
    outs = [r["out"] for r in res.results]
    b_eff = (np.asarray(b_proj, np.float32)
             + np.asarray(w_proj, np.float32) @ np.asarray(b_attn, np.float32)[2 * C:])
    full = np.stack(
        [sum(outs[b * CPG:(b + 1) * CPG][1:], outs[b * CPG]) + b_eff
         for b in range(B)]
    ).astype(np.float32)
    return full, res


def kernel(x, w_attn, b_attn, w_proj, b_proj):
    out, _ = run_device(x, w_attn, b_attn, w_proj, b_proj)
    return out
